# revision 2
# baseline (speedup 1.0000x reference)
"""Trainium2 Bass kernel for nn_EntInit (gnn_message_passing).

feat[n, :] = mean over incoming edges e (dst[e] == n) of T[etypes[e], :]
where T = concat(rel_head_emb, rel_tail_emb)  [400, 128].

Formulation: the per-(type, node) incidence histogram A[t, n] (small
integer counts, max ~8 for this edge distribution) is assembled on the
host with a single bincount -- pure index bookkeeping, no FLOPs -- and
shipped node-sharded to the 8 cores as bf16 (exact for ints < 256).
All dense math runs on device: for each 128-node block,
  feat_blk = (A_blk^T @ [T_hi | T_lo]) * rcp_blk
with the f32 table split into bf16 hi/lo halves so products stay exact
in f32 PSUM, and rcp = 1/max(count, 1) precomputed exactly on host
(counts = bincount(dst)).  Per (block, type-chunk of 100): one hi and
one lo matmul accumulate into PSUM; eviction fuses the mean divide
(per-partition scalar multiply) alternating between the Scalar and
Vector engines; output DMAs issue from the Pool queue.  A streams in
seven double-buffered chunks so the PE starts ~2us in.
"""
import sys

sys.path.insert(0, "/opt/trn_rl_repo")

import numpy as np
import ml_dtypes

import concourse.bass as bass
import concourse.bacc as bacc
import concourse.mybir as mybir
import concourse.tile as tile

NUM_REL = 200
N_TYPES = 2 * NUM_REL          # 400 relation rows
N_CORES = 8
P = 128
WC = 100                       # type-chunk width (4 chunks x 100 = 400)
NCH = 4
BF16 = ml_dtypes.bfloat16

# --- tuning flags ---
LO_PASS = True                 # include bf16 lo-residual table pass
OUT_BF16 = False               # write feat as bf16, upcast on host
A_FP8 = False                  # ship A as fp8e4 (exact ints <= 16)

_prog_cache: dict = {}
_runner_cache: dict = {}


def _build_program(B: int, repeats: int = 1,
                   lo=LO_PASS, out_bf16=OUT_BF16, a_fp8=A_FP8):
    """One SPMD program; cores differ only in input data.
    B node-blocks of 128 nodes per core."""
    TTn = B * NCH * P          # free dim of A per core
    TW = 256 if lo else 128    # per-chunk table width (hi | lo)
    a_dt = mybir.dt.float8e4 if a_fp8 else mybir.dt.bfloat16
    o_dt = mybir.dt.bfloat16 if out_bf16 else mybir.dt.float32
    nc = bacc.Bacc("TRN2", debug=False, num_devices=1)
    a = nc.dram_tensor("a", [WC, TTn], a_dt, kind="ExternalInput").ap()
    tbl = nc.dram_tensor("tbl", [WC, NCH * TW], a_dt,
                         kind="ExternalInput").ap()
    rcp = nc.dram_tensor("rcp", [P, B], mybir.dt.float32,
                         kind="ExternalInput").ap()
    feat = nc.dram_tensor("feat", [B * P, P], o_dt,
                          kind="ExternalOutput").ap()

    # chunking of the A stream: CB blocks per chunk
    CB = 7
    NCHK = -(-B // CB)

    with tile.TileContext(nc) as tc:
        with (
            tc.tile_pool(name="const", bufs=1) as const_tp,
            tc.tile_pool(name="ach", bufs=3) as a_tp,
            tc.tile_pool(name="ft", bufs=4) as ft_tp,
            tc.tile_pool(name="ps", bufs=4, space="PSUM") as ps_tp,
        ):
            tbl_sb = const_tp.tile([WC, NCH, TW], a_dt)
            rcp_sb = const_tp.tile([P, B], mybir.dt.float32)
            nc.scalar.dma_start(out=tbl_sb[:], in_=tbl[:])
            nc.scalar.dma_start(out=rcp_sb[:], in_=rcp[:])

            for _rep in range(repeats):
                for ch in range(NCHK):
                    b0 = ch * CB
                    nb = min(CB, B - b0)
                    a_sb = a_tp.tile([WC, CB * NCH * P], a_dt, tag="a")
                    nc.sync.dma_start(
                        out=a_sb[:, 0:nb * NCH * P],
                        in_=a[:, b0 * NCH * P:(b0 + nb) * NCH * P])
                    for bb in range(nb):
                        b = b0 + bb
                        ps = ps_tp.tile([P, P], mybir.dt.float32, tag="ps")
                        for c in range(NCH):
                            off = (bb * NCH + c) * P
                            nc.tensor.matmul(
                                out=ps[:], lhsT=a_sb[:, off:off + P],
                                rhs=tbl_sb[:, c, 0:128],
                                start=(c == 0),
                                stop=(c == NCH - 1 and not lo))
                            if lo:
                                nc.tensor.matmul(
                                    out=ps[:], lhsT=a_sb[:, off:off + P],
                                    rhs=tbl_sb[:, c, 128:256],
                                    start=False, stop=(c == NCH - 1))
                        ft = ft_tp.tile([P, P], o_dt, tag="ft")
                        if b % 2 == 0:
                            nc.scalar.mul(out=ft[:], in_=ps[:],
                                          mul=rcp_sb[:, b:b + 1])
                        else:
                            nc.vector.tensor_scalar(
                                out=ft[:], in0=ps[:],
                                scalar1=rcp_sb[:, b:b + 1], scalar2=None,
                                op0=mybir.AluOpType.mult)
                        nc.gpsimd.dma_start(
                            out=feat[b * P:(b + 1) * P, :], in_=ft[:])

    nc.compile()
    return nc


def _host_prepare(et: np.ndarray, d: np.ndarray,
                  head: np.ndarray, tail: np.ndarray, nn: int,
                  lo=LO_PASS, a_fp8=A_FP8):
    """Build concatenated (over cores) device inputs.

    Returns dict name -> np.ndarray shaped [N_CORES * p0, ...], plus B.
    """
    B = -(-(-(-nn // P)) // N_CORES)   # blocks per core
    npc = B * P                        # nodes per core
    npad = npc * N_CORES

    cnt = np.bincount(d, minlength=npad)
    rcp = (1.0 / np.maximum(cnt, 1.0)).astype(np.float32)
    rcp_cat = np.ascontiguousarray(
        rcp.reshape(N_CORES, B, P).transpose(0, 2, 1).reshape(N_CORES * P, B))

    # A histogram directly in device layout:
    #   row = core * WC + (et % WC)
    #   col = blk * (NCH * P) + (et // WC) * P + (node % P)
    core, r = np.divmod(d, npc)
    blk, p = np.divmod(r, P)
    c, tl = np.divmod(et, WC)
    TTn = B * NCH * P
    key = (core * WC + tl) * TTn + blk * (NCH * P) + c * P + p
    acnt = np.bincount(key, minlength=N_CORES * WC * TTn)
    amax = int(acnt.max())
    a_np = ml_dtypes.float8_e4m3fn if a_fp8 else BF16
    a_cat = acnt.astype(np.float32).astype(a_np).reshape(N_CORES * WC, TTn)

    W = np.concatenate([head, tail], axis=0).astype(np.float32)
    hi = W.astype(a_np)
    TW = 256 if lo else 128
    tbl = np.zeros((WC, NCH, TW), a_np)
    for cc in range(NCH):
        tbl[:, cc, 0:128] = hi[cc * WC:(cc + 1) * WC]
        if lo:
            tbl[:, cc, 128:256] = (W[cc * WC:(cc + 1) * WC]
                                   - hi[cc * WC:(cc + 1) * WC]
                                   .astype(np.float32)).astype(a_np)
    tbl_cat = np.tile(tbl.reshape(WC, NCH * TW), (N_CORES, 1))

    ins = {"a": a_cat, "tbl": tbl_cat, "rcp": rcp_cat}
    return ins, B, amax


def _get_runner(nc):
    """Cached jitted SPMD executor."""
    key = id(nc)
    if key in _runner_cache:
        return _runner_cache[key]
    import jax
    from jax.experimental.shard_map import shard_map
    from jax.sharding import Mesh, PartitionSpec
    from concourse import bass2jax
    from concourse.bass2jax import _bass_exec_p, partition_id_tensor

    bass2jax.install_neuronx_cc_hook()

    in_names, out_names, out_avals, zero_shapes = [], [], [], []
    for alloc in nc.m.functions[0].allocations:
        if not isinstance(alloc, mybir.MemoryLocationSet):
            continue
        name = alloc.memorylocations[0].name
        if alloc.kind == "ExternalInput":
            if nc.partition_id_tensor is None or name != nc.partition_id_tensor.name:
                in_names.append(name)
        elif alloc.kind == "ExternalOutput":
            shape = tuple(alloc.tensor_shape)
            dtype = mybir.dt.np(alloc.dtype)
            out_names.append(name)
            out_avals.append(jax.core.ShapedArray(shape, dtype))
            zero_shapes.append((shape, dtype))
    n_params = len(in_names)
    all_names = list(in_names) + list(out_names)
    if nc.partition_id_tensor is not None:
        all_names.append(nc.partition_id_tensor.name)
    donate = tuple(range(n_params, n_params + len(out_names)))

    def _body(*args):
        operands = list(args)
        if nc.partition_id_tensor is not None:
            operands.append(partition_id_tensor())
        outs = _bass_exec_p.bind(
            *operands,
            out_avals=tuple(out_avals),
            in_names=tuple(all_names),
            out_names=tuple(out_names),
            lowering_input_output_aliases=(),
            sim_require_finite=True,
            sim_require_nnan=True,
            nc=nc,
        )
        return tuple(outs)

    devices = jax.devices()[:N_CORES]
    mesh = Mesh(np.asarray(devices), ("core",))
    in_specs = (PartitionSpec("core"),) * (n_params + len(out_names))
    out_specs = (PartitionSpec("core"),) * len(out_names)
    fn = jax.jit(
        shard_map(_body, mesh=mesh, in_specs=in_specs, out_specs=out_specs,
                  check_rep=False),
        donate_argnums=donate, keep_unused=True,
    )
    r = (fn, in_names, out_names, out_avals, zero_shapes)
    _runner_cache[key] = r
    return r


def _run_concat(nc, ins: dict):
    """Run the SPMD program on concatenated inputs; returns dict of
    concatenated outputs."""
    fn, in_names, out_names, out_avals, zero_shapes = _get_runner(nc)
    concat_in = [ins[n] for n in in_names]
    concat_zeros = [np.zeros((N_CORES * s[0], *s[1:]), dt)
                    for s, dt in zero_shapes]
    out_arrs = fn(*concat_in, *concat_zeros)
    return {name: np.asarray(out_arrs[i]) for i, name in enumerate(out_names)}


def kernel(etypes, dst, rel_head_emb, rel_tail_emb, n_nodes):
    et = np.asarray(etypes).astype(np.int64)
    d = np.asarray(dst).astype(np.int64)
    head = np.asarray(rel_head_emb, dtype=np.float32)
    tail = np.asarray(rel_tail_emb, dtype=np.float32)
    nn = int(n_nodes)

    ins, B, amax = _host_prepare(et, d, head, tail, nn)
    if amax > (16 if A_FP8 else 256):
        # pathological duplicate-edge density: exact host fallback
        W = np.concatenate([head, tail], axis=0)
        A = np.bincount(d * N_TYPES + et, minlength=nn * N_TYPES)
        A = A.reshape(nn, N_TYPES).astype(np.float32)
        cntf = A.sum(axis=1)
        return (A @ W) / np.maximum(cntf, 1.0)[:, None]

    key = (B, 1, LO_PASS, OUT_BF16, A_FP8)
    if key not in _prog_cache:
        _prog_cache[key] = _build_program(B)
    nc = _prog_cache[key]

    import time as _time
    _t0 = _time.perf_counter()
    outs = _run_concat(nc, ins)
    global LAST_DEVICE_WALL
    LAST_DEVICE_WALL = _time.perf_counter() - _t0

    out = outs["feat"]
    if out.dtype != np.float32:
        out = out.astype(np.float32)
    return out[:nn]


# revision 3
# speedup vs baseline: 39.3164x; 39.3164x over previous
"""Trainium2 Bass kernel for nn_EntInit (gnn_message_passing).

feat[n, :] = mean over incoming edges e (dst[e] == n) of T[etypes[e], :]
where T = concat(rel_head_emb, rel_tail_emb)  [400, 128].

Formulation: the per-(type, node) incidence histogram A[t, n] (small
integer counts, max ~8 for this edge distribution) is assembled on the
host with a single bincount -- pure index bookkeeping, no FLOPs -- and
shipped node-sharded to the 8 cores as bf16 (exact for ints < 256).
All dense math runs on device: for each 128-node block,
  feat_blk = (A_blk^T @ [T_hi | T_lo]) * rcp_blk
with the f32 table split into bf16 hi/lo halves so products stay exact
in f32 PSUM, and rcp = 1/max(count, 1) precomputed exactly on host
(counts = bincount(dst)).  Per (block, type-chunk of 100): one hi and
one lo matmul accumulate into PSUM; eviction fuses the mean divide
(per-partition scalar multiply) alternating between the Scalar and
Vector engines; output DMAs issue from the Pool queue.  A streams in
seven double-buffered chunks so the PE starts ~2us in.
"""
import sys

sys.path.insert(0, "/opt/trn_rl_repo")

import numpy as np
import ml_dtypes

import concourse.bass as bass
import concourse.bacc as bacc
import concourse.mybir as mybir
import concourse.tile as tile

NUM_REL = 200
N_TYPES = 2 * NUM_REL          # 400 relation rows
N_CORES = 8
P = 128
WC = 100                       # type-chunk width (4 chunks x 100 = 400)
NCH = 4
BF16 = ml_dtypes.bfloat16

# --- tuning flags ---
LO_PASS = True                 # include bf16 lo-residual table pass
OUT_BF16 = False               # write feat as bf16, upcast on host
A_FP8 = False                  # ship A as fp8e4 (exact ints <= 16)

_prog_cache: dict = {}
_runner_cache: dict = {}


def _build_program(B: int, repeats: int = 1,
                   lo=LO_PASS, out_bf16=OUT_BF16, a_fp8=A_FP8,
                   twin: str = "full"):
    """One SPMD program; cores differ only in input data.
    B node-blocks of 128 nodes per core.

    twin: "full" (normal), "nodma" (A resident, reps only compute+out),
    "dmaonly" (reps only stream A, no compute)."""
    TTn = B * NCH * P          # free dim of A per core
    TW = 256 if lo else 128    # per-chunk table width (hi | lo)
    a_dt = mybir.dt.float8e4 if a_fp8 else mybir.dt.bfloat16
    o_dt = mybir.dt.bfloat16 if out_bf16 else mybir.dt.float32
    nc = bacc.Bacc("TRN2", debug=False, num_devices=1)
    a = nc.dram_tensor("a", [WC, TTn], a_dt, kind="ExternalInput").ap()
    tbl = nc.dram_tensor("tbl", [WC, NCH * TW], a_dt,
                         kind="ExternalInput").ap()
    rcp = nc.dram_tensor("rcp", [P, B], mybir.dt.float32,
                         kind="ExternalInput").ap()
    feat = nc.dram_tensor("feat", [B * P, P], o_dt,
                          kind="ExternalOutput").ap()

    # chunking of the A stream: CB blocks per chunk
    CB = 7
    NCHK = -(-B // CB)

    with tile.TileContext(nc) as tc:
        with (
            tc.tile_pool(name="const", bufs=1) as const_tp,
            tc.tile_pool(name="ach", bufs=1 if twin == "nodma" else 3) as a_tp,
            tc.tile_pool(name="ft", bufs=4) as ft_tp,
            tc.tile_pool(name="ps", bufs=4, space="PSUM") as ps_tp,
        ):
            tbl_sb = const_tp.tile([WC, NCH, TW], a_dt)
            rcp_sb = const_tp.tile([P, B], mybir.dt.float32)
            nc.scalar.dma_start(out=tbl_sb[:], in_=tbl[:])
            nc.scalar.dma_start(out=rcp_sb[:], in_=rcp[:])

            if twin == "nodma":
                a_all = a_tp.tile([WC, TTn], a_dt)
                nc.sync.dma_start(out=a_all[:], in_=a[:])

            def compute_block(a_sb, off, b):
                ps = ps_tp.tile([P, P], mybir.dt.float32, tag="ps")
                for c in range(NCH):
                    o = off + c * P
                    nc.tensor.matmul(
                        out=ps[:], lhsT=a_sb[:, o:o + P],
                        rhs=tbl_sb[:, c, 0:128],
                        start=(c == 0),
                        stop=(c == NCH - 1 and not lo))
                    if lo:
                        nc.tensor.matmul(
                            out=ps[:], lhsT=a_sb[:, o:o + P],
                            rhs=tbl_sb[:, c, 128:256],
                            start=False, stop=(c == NCH - 1))
                ft = ft_tp.tile([P, P], o_dt, tag="ft")
                if b % 2 == 0:
                    nc.scalar.mul(out=ft[:], in_=ps[:],
                                  mul=rcp_sb[:, b:b + 1])
                else:
                    nc.vector.tensor_scalar(
                        out=ft[:], in0=ps[:],
                        scalar1=rcp_sb[:, b:b + 1], scalar2=None,
                        op0=mybir.AluOpType.mult)
                nc.gpsimd.dma_start(
                    out=feat[b * P:(b + 1) * P, :], in_=ft[:])

            for _rep in range(repeats):
                if twin == "nodma":
                    for b in range(B):
                        compute_block(a_all, b * NCH * P, b)
                    continue
                for ch in range(NCHK):
                    b0 = ch * CB
                    nb = min(CB, B - b0)
                    a_sb = a_tp.tile([WC, CB * NCH * P], a_dt, tag="a")
                    nc.sync.dma_start(
                        out=a_sb[:, 0:nb * NCH * P],
                        in_=a[:, b0 * NCH * P:(b0 + nb) * NCH * P])
                    if twin == "dmaonly":
                        continue
                    for bb in range(nb):
                        compute_block(a_sb, bb * NCH * P, b0 + bb)

    nc.compile()
    return nc


def _host_prepare(et: np.ndarray, d: np.ndarray,
                  head: np.ndarray, tail: np.ndarray, nn: int,
                  lo=LO_PASS, a_fp8=A_FP8):
    """Build concatenated (over cores) device inputs.

    Returns dict name -> np.ndarray shaped [N_CORES * p0, ...], plus B.
    """
    B = -(-(-(-nn // P)) // N_CORES)   # blocks per core
    npc = B * P                        # nodes per core
    npad = npc * N_CORES

    cnt = np.bincount(d, minlength=npad)
    rcp = (1.0 / np.maximum(cnt, 1.0)).astype(np.float32)
    rcp_cat = np.ascontiguousarray(
        rcp.reshape(N_CORES, B, P).transpose(0, 2, 1).reshape(N_CORES * P, B))

    # A histogram directly in device layout:
    #   row = core * WC + (et % WC)
    #   col = blk * (NCH * P) + (et // WC) * P + (node % P)
    core, r = np.divmod(d, npc)
    blk, p = np.divmod(r, P)
    c, tl = np.divmod(et, WC)
    TTn = B * NCH * P
    key = (core * WC + tl) * TTn + blk * (NCH * P) + c * P + p
    acnt = np.bincount(key, minlength=N_CORES * WC * TTn)
    amax = int(acnt.max())
    a_np = ml_dtypes.float8_e4m3fn if a_fp8 else BF16
    a_cat = acnt.astype(np.float32).astype(a_np).reshape(N_CORES * WC, TTn)

    W = np.concatenate([head, tail], axis=0).astype(np.float32)
    hi = W.astype(a_np)
    TW = 256 if lo else 128
    tbl = np.zeros((WC, NCH, TW), a_np)
    for cc in range(NCH):
        tbl[:, cc, 0:128] = hi[cc * WC:(cc + 1) * WC]
        if lo:
            tbl[:, cc, 128:256] = (W[cc * WC:(cc + 1) * WC]
                                   - hi[cc * WC:(cc + 1) * WC]
                                   .astype(np.float32)).astype(a_np)
    tbl_cat = np.tile(tbl.reshape(WC, NCH * TW), (N_CORES, 1))

    ins = {"a": a_cat, "tbl": tbl_cat, "rcp": rcp_cat}
    return ins, B, amax


def _get_runner(nc):
    """Cached jitted SPMD executor."""
    key = id(nc)
    if key in _runner_cache:
        return _runner_cache[key]
    import jax
    from jax.experimental.shard_map import shard_map
    from jax.sharding import Mesh, PartitionSpec
    from concourse import bass2jax
    from concourse.bass2jax import _bass_exec_p, partition_id_tensor

    bass2jax.install_neuronx_cc_hook()

    in_names, out_names, out_avals, zero_shapes = [], [], [], []
    for alloc in nc.m.functions[0].allocations:
        if not isinstance(alloc, mybir.MemoryLocationSet):
            continue
        name = alloc.memorylocations[0].name
        if alloc.kind == "ExternalInput":
            if nc.partition_id_tensor is None or name != nc.partition_id_tensor.name:
                in_names.append(name)
        elif alloc.kind == "ExternalOutput":
            shape = tuple(alloc.tensor_shape)
            dtype = mybir.dt.np(alloc.dtype)
            out_names.append(name)
            out_avals.append(jax.core.ShapedArray(shape, dtype))
            zero_shapes.append((shape, dtype))
    n_params = len(in_names)
    all_names = list(in_names) + list(out_names)
    if nc.partition_id_tensor is not None:
        all_names.append(nc.partition_id_tensor.name)
    donate = tuple(range(n_params, n_params + len(out_names)))

    def _body(*args):
        operands = list(args)
        if nc.partition_id_tensor is not None:
            operands.append(partition_id_tensor())
        outs = _bass_exec_p.bind(
            *operands,
            out_avals=tuple(out_avals),
            in_names=tuple(all_names),
            out_names=tuple(out_names),
            lowering_input_output_aliases=(),
            sim_require_finite=True,
            sim_require_nnan=True,
            nc=nc,
        )
        return tuple(outs)

    devices = jax.devices()[:N_CORES]
    mesh = Mesh(np.asarray(devices), ("core",))
    in_specs = (PartitionSpec("core"),) * (n_params + len(out_names))
    out_specs = (PartitionSpec("core"),) * len(out_names)
    fn = jax.jit(
        shard_map(_body, mesh=mesh, in_specs=in_specs, out_specs=out_specs,
                  check_rep=False),
        donate_argnums=donate, keep_unused=True,
    )
    r = (fn, in_names, out_names, out_avals, zero_shapes)
    _runner_cache[key] = r
    return r


def _run_concat(nc, ins: dict):
    """Run the SPMD program on concatenated inputs; returns dict of
    concatenated outputs."""
    fn, in_names, out_names, out_avals, zero_shapes = _get_runner(nc)
    concat_in = [ins[n] for n in in_names]
    concat_zeros = [np.zeros((N_CORES * s[0], *s[1:]), dt)
                    for s, dt in zero_shapes]
    out_arrs = fn(*concat_in, *concat_zeros)
    return {name: np.asarray(out_arrs[i]) for i, name in enumerate(out_names)}


def kernel(etypes, dst, rel_head_emb, rel_tail_emb, n_nodes):
    et = np.asarray(etypes).astype(np.int64)
    d = np.asarray(dst).astype(np.int64)
    head = np.asarray(rel_head_emb, dtype=np.float32)
    tail = np.asarray(rel_tail_emb, dtype=np.float32)
    nn = int(n_nodes)

    ins, B, amax = _host_prepare(et, d, head, tail, nn)
    if amax > (16 if A_FP8 else 256):
        # pathological duplicate-edge density: exact host fallback
        W = np.concatenate([head, tail], axis=0)
        A = np.bincount(d * N_TYPES + et, minlength=nn * N_TYPES)
        A = A.reshape(nn, N_TYPES).astype(np.float32)
        cntf = A.sum(axis=1)
        return (A @ W) / np.maximum(cntf, 1.0)[:, None]

    key = (B, 1, LO_PASS, OUT_BF16, A_FP8)
    if key not in _prog_cache:
        _prog_cache[key] = _build_program(B)
    nc = _prog_cache[key]

    import time as _time
    _t0 = _time.perf_counter()
    outs = _run_concat(nc, ins)
    global LAST_DEVICE_WALL
    LAST_DEVICE_WALL = _time.perf_counter() - _t0

    out = outs["feat"]
    if out.dtype != np.float32:
        out = out.astype(np.float32)
    return out[:nn]
